# revision 20
# baseline (speedup 1.0000x reference)
"""2-layer GAT (GATConv x2 + LayerNorm + ReLU) on Trainium2, 8-core SPMD.

v2 design (vs baseline): pair-gather edge streams.
  - Nodes degree-sorted (incl self loop), dealt round-robin across 8 cores;
    each core owns NPC=6272 dst slots (49 tiles x 128).
  - Per layer a packed bf16 node table in DRAM:
      layer 1: row = [h(128) | a_s(4) | a_d(4) | pad] -> 192 cols (384B)
      layer 2: row = [h2(64) | a_s2(1) | a_d2(1) | pad] -> 128 cols (256B)
    Rows are gathered in PAIRS (idx = row>>1, elem = 2 rows) so a single
    int16 gather covers all 50176 rows (25088 pair indices < 32768): ONE
    dma_gather per dst tile per layer, real edges only (~1.06x padding
    vs 1.86x for the per-half slot-padded baseline).
  - Per dst tile [128 dst x K slots x 2 pair-members]: slot 0 is the self
    loop; host-built bf16 masks (-1e9 on the wrong pair member / pad slots)
    zero out garbage in the softmax.  e-scores are bounded (|e| <= ~6) so
    the segment max is skipped; weights w = exp(leaky(e))/den are folded
    into the gathered h before a single strided DVE reduction.
  - Layer-2 tables AllGathered in 4 chunks (Shared scratchpad output)
    overlapped with the layer-1 tail.
"""

import os
import types
from contextlib import ExitStack

import numpy as np

import concourse.bass as bass
import concourse.mybir as mybir
import concourse.tile as tile
from concourse import bacc
from concourse.bass import AP
from concourse.masks import make_identity

F32 = mybir.dt.float32
BF16 = mybir.dt.bfloat16
I16 = mybir.dt.int16
AX = mybir.AxisListType
OP = mybir.AluOpType
ACT = mybir.ActivationFunctionType

# ---------------------------------------------------------------- problem cfg
N = 50000
E = 800000
IN_DIM = 128
HID = 32
HEADS = 4
EMB = 64
NEG = 0.2
EPS = 1e-5
NCORE = 8
ROW1 = 192        # layer-1 packed table row (bf16 cols): h(128) as(4) ad(4) pad
ROW2 = 128        # layer-2 packed table row: h2(64) as(1) ad(1) pad
NEGBIG = -1e9
AG_CHUNKS = 4


def make_cfg(n_nodes, tiles_per_core):
    c = types.SimpleNamespace()
    c.N = n_nodes
    c.TILES = tiles_per_core
    c.NPC = tiles_per_core * 128
    c.NPAD = NCORE * c.NPC
    c.NPAIR = c.NPAD // 2
    c.REAL_PC = n_nodes // NCORE
    # AllGather chunk geometry (tile index ranges per chunk)
    ends = [((i + 1) * tiles_per_core) // AG_CHUNKS for i in range(AG_CHUNKS)]
    starts = [0] + ends[:-1]
    c.AG_T0 = starts
    c.AG_T1 = ends
    # table rows are stored [chunk, core, rows] so each chunked AllGather
    # output region is contiguous; srow maps (core, pos) -> storage row
    rs = [s * 128 for s in starts]
    re = [e * 128 for e in ends]
    c.CH_ROWS = [b - a for a, b in zip(rs, re)]
    c.CH_BASE = [NCORE * a for a in rs]
    pos = np.arange(c.NPC)
    k = np.searchsorted(np.asarray(re), pos, side="right")
    c.SROW = np.stack([
        np.asarray(c.CH_BASE)[k] + cc * np.asarray(c.CH_ROWS)[k]
        + (pos - np.asarray(rs)[k]) for cc in range(NCORE)])  # [core, pos]
    assert c.REAL_PC % 2 == 0 and c.REAL_PC < c.NPC and c.NPAIR <= 32768
    return c


CFG = make_cfg(N, 49)


# ------------------------------------------------------------------ host prep
def host_prep(cfg, edge_index):
    """Node permutation, per-tile slot counts, idx16 pair stream and masks.

    Per (core, tile t) the gather stream is K[t] columns of 128 slots:
    column 0 = self loops, columns 1.. = neighbors (dst-grouped), pads point
    at the PADPAIR rows.  idx values are PAIR indices (row>>1); the bf16
    mask [128, 2*K[t]] holds 0 on the real pair member and -1e9 elsewhere.
    """
    n, npc, tiles = cfg.N, cfg.NPC, cfg.TILES
    src = np.asarray(edge_index[0], np.int64)
    dst = np.asarray(edge_index[1], np.int64)

    deg = np.bincount(dst, minlength=n) + 1           # incl self loop
    order = np.argsort(-deg, kind="stable")
    newid = np.empty(n, np.int64)
    r = np.arange(n)
    newid[order] = (r % NCORE) * npc + (r // NCORE)
    new2old = np.full(NCORE * npc, -1, np.int64)
    new2old[newid] = np.arange(n)

    degs_sorted = np.zeros(tiles * 1024, np.int64)
    degs_sorted[:n] = deg[order]
    K = degs_sorted.reshape(tiles, 1024).max(axis=1)
    K = np.maximum(K, 1).astype(np.int64)             # slots per tile

    # group consecutive tiles (equal slot count S per group) for batched
    # gathers + DVE ops; n*S <= GCAP slots, n <= 4 (psum bank limit)
    GCAP = 64
    ag_ends = set(cfg.AG_T1)
    groups = []
    t = 0
    while t < tiles:
        S = int(K[t])
        ng = 1
        while (t + ng < tiles and ng < 4
               and (t + ng) not in ag_ends
               and (ng + 1) * max(S, int(K[t + ng])) <= GCAP):
            S = max(S, int(K[t + ng]))
            ng += 1
        groups.append((t, ng, S))
        t += ng
    gslots = [ng * S for (_, ng, S) in groups]
    GOFS = [0]
    for gs in gslots:
        GOFS.append(GOFS[-1] + gs * 128)              # idx stream offsets
    MOFS_G = [0]
    for gs in gslots:
        MOFS_G.append(MOFS_G[-1] + gs * 2)            # mask col offsets
    total_idx = GOFS[-1]
    mtot = MOFS_G[-1]
    # per-tile lookup: group id, index within group, group S
    g_of_t = np.zeros(tiles, np.int64)
    i_of_t = np.zeros(tiles, np.int64)
    S_of_t = np.zeros(tiles, np.int64)
    for gi, (t0, ng, S) in enumerate(groups):
        g_of_t[t0:t0 + ng] = gi
        i_of_t[t0:t0 + ng] = np.arange(ng)
        S_of_t[t0:t0 + ng] = S
    GOFS_np = np.asarray(GOFS[:-1])
    MOFS_np = np.asarray(MOFS_G[:-1])

    tl = cfg.SROW.reshape(-1)              # logical row -> storage row
    padpair = int(tl[cfg.REAL_PC]) >> 1    # core-0 pad rows (zero features)
    assert int(tl[cfg.REAL_PC]) % 2 == 0

    ns, nd = newid[src], newid[dst]
    eo = np.argsort(nd, kind="stable")
    snd, sns = nd[eo], ns[eo]
    starts = np.r_[0, np.flatnonzero(np.diff(snd)) + 1]
    runlen = np.diff(np.r_[starts, len(snd)])
    runpos = np.arange(len(snd)) - np.repeat(starts, runlen)
    slot = runpos + 1                                  # col 0 = self
    cs = snd // npc
    pos = snd % npc
    ts_ = pos // 128
    ps_ = pos % 128
    assert (slot < K[ts_]).all()

    srows = tl[sns]
    idx16 = np.full((NCORE, total_idx), padpair, np.int16)
    mask = np.full((NCORE, 128, mtot), NEGBIG, np.float32)
    scol = i_of_t[ts_] * S_of_t[ts_] + slot           # slot col within group
    posi = GOFS_np[g_of_t[ts_]] + scol * 128 + ps_
    idx16[cs, posi] = (srows >> 1).astype(np.int16)
    mask[cs, ps_, MOFS_np[g_of_t[ts_]] + scol * 2 + (srows & 1)] = 0.0

    # self column (slot 0 of each tile) per core
    for c in range(NCORE):
        own = c * npc + np.arange(npc)
        valid = new2old[own] >= 0
        t_all = np.arange(npc) // 128
        p_all = np.arange(npc) % 128
        sc0 = i_of_t[t_all] * S_of_t[t_all]
        sown = tl[own]
        idx16[c, GOFS_np[g_of_t[t_all[valid]]] + sc0[valid] * 128
              + p_all[valid]] = (sown[valid] >> 1).astype(np.int16)
        mask[c, p_all[valid], MOFS_np[g_of_t[t_all[valid]]] + sc0[valid] * 2
             + (sown[valid] & 1)] = 0.0

    w = idx16.reshape(NCORE, total_idx // 16, 16).transpose(0, 2, 1)
    idx16_w = np.ascontiguousarray(np.tile(w, (1, 8, 1)))
    return types.SimpleNamespace(
        new2old=new2old, newid=newid,
        K=[int(v) for v in K], groups=groups,
        GOFS=[v // 16 for v in GOFS], MOFS=MOFS_G,
        c16=total_idx // 16, mtot=mtot,
        idx16=idx16_w, mask=_bf16(mask),
    )


def _bf16(x):
    import ml_dtypes
    x = np.ascontiguousarray(np.asarray(x, np.float32))
    u = x.view(np.uint32)
    r = ((u + 0x7FFF + ((u >> 16) & 1)) >> 16).astype(np.uint16)
    return r.view(ml_dtypes.bfloat16)


def host_weights(cfg, inputs):
    W1 = np.asarray(inputs["W1"], np.float32)
    W2 = np.asarray(inputs["W2"], np.float32)
    as1 = np.asarray(inputs["att_src1"], np.float32)
    ad1 = np.asarray(inputs["att_dst1"], np.float32)
    as2 = np.asarray(inputs["att_src2"], np.float32)
    ad2 = np.asarray(inputs["att_dst2"], np.float32)
    W1r = W1.reshape(IN_DIM, HEADS, HID)
    w_as1 = np.einsum("fhc,hc->fh", W1r, as1)
    w_ad1 = np.einsum("fhc,hc->fh", W1r, ad1)
    W1ext = np.concatenate([W1, w_as1, w_ad1], axis=1)            # [128,136]
    W2ext = np.concatenate([W2, W2 @ as2[0][:, None], W2 @ ad2[0][:, None]],
                           axis=1)                                # [128,66]
    par0 = ((np.arange(128) + 1) % 2).astype(np.float32)[:, None]
    par1 = (np.arange(128) % 2).astype(np.float32)[:, None]
    return {
        "w1ext": _bf16(W1ext), "w2ext": _bf16(W2ext),
        "b1": np.tile(np.asarray(inputs["b1"], np.float32), (128, 1)),
        "g1": np.tile(np.asarray(inputs["gamma1"], np.float32), (128, 1)),
        "be1": np.tile(np.asarray(inputs["beta1"], np.float32), (128, 1)),
        "b2": np.tile(np.asarray(inputs["b2"], np.float32), (128, 1)),
        "g2": np.tile(np.asarray(inputs["gamma2"], np.float32), (128, 1)),
        "be2": np.tile(np.asarray(inputs["beta2"], np.float32), (128, 1)),
        "par0": par0, "par1": par1,
    }


def host_xt(cfg, prep, x):
    tl = cfg.SROW.reshape(-1)
    xt = np.zeros((IN_DIM, cfg.NPAD), np.float32)
    xt[:, tl[prep.newid]] = np.asarray(x, np.float32).T
    return _bf16(xt)


# ----------------------------------------------------------------- AP helpers
def apv(ap: AP, dims, extra_offset=0):
    """Replace the free dims of `ap` with explicit [step, count] pairs."""
    return AP(ap.tensor, int(ap.offset + extra_offset),
              [list(ap.ap[0])] + [[int(s), int(n)] for s, n in dims])


def apd(ap: AP, dims, extra_offset=0):
    """DRAM AP with explicit dims (no partition dim)."""
    return AP(ap.tensor, int(ap.offset + extra_offset),
              [[int(s), int(n)] for s, n in dims])


# ------------------------------------------------------------- device program
def build_program(cfg, prep):
    nc = bacc.Bacc("TRN2", target_bir_lowering=False, debug=False,
                   num_devices=NCORE)
    tiles, npc, npad = cfg.TILES, cfg.NPC, cfg.NPAD
    GOFS, MOFS = prep.GOFS, prep.MOFS
    FB1 = IN_DIM + 2 * HEADS            # 136
    FB2 = EMB + 2                       # 66

    XT = nc.dram_tensor("xt", [IN_DIM, npad], BF16, kind="ExternalInput")
    W1e = nc.dram_tensor("w1ext", [IN_DIM, FB1], BF16, kind="ExternalInput")
    W2e = nc.dram_tensor("w2ext", [IN_DIM, FB2], BF16, kind="ExternalInput")
    IDX16 = nc.dram_tensor("idx16", [128, prep.c16], I16, kind="ExternalInput")
    MASK = nc.dram_tensor("mask", [128, prep.mtot], BF16, kind="ExternalInput")
    CB = {}
    for nm, cols in [("b1", IN_DIM), ("g1", IN_DIM), ("be1", IN_DIM),
                     ("b2", EMB), ("g2", EMB), ("be2", EMB),
                     ("par0", 1), ("par1", 1)]:
        CB[nm] = nc.dram_tensor(nm, [128, cols], F32, kind="ExternalInput")
    OUT = nc.dram_tensor("out", [npc, EMB], F32, kind="ExternalOutput")

    with tile.TileContext(nc, num_cores=NCORE) as tc, ExitStack() as ctx:
        dram = ctx.enter_context(tc.tile_pool(name="dram", bufs=1,
                                              space="DRAM"))
        t1b = dram.tile([npad, ROW1], BF16, name="t1b")
        t2sh = dram.tile([npc, ROW2], BF16, name="t2sh")
        t2b = dram.tile([npad, ROW2], BF16, name="t2b")

        cpool = ctx.enter_context(tc.tile_pool(name="const", bufs=1))
        w1s = cpool.tile([IN_DIM, FB1], BF16, name="w1s")
        w2s = cpool.tile([IN_DIM, FB2], BF16, name="w2s")
        nc.sync.dma_start(w1s[:], W1e[:])
        nc.sync.dma_start(w2s[:], W2e[:])
        cb = {}
        for nm in CB:
            cb[nm] = cpool.tile(list(CB[nm].shape), F32, name=f"sb_{nm}")
            nc.sync.dma_start(cb[nm][:], CB[nm][:])
        ident = cpool.tile([128, 128], F32, name="ident")
        make_identity(nc, ident[:])
        epst = cpool.tile([128, 1], F32, name="epst")
        nc.vector.memset(epst[:], EPS)
        i16b = cpool.tile([128, prep.c16], I16, name="i16b")
        nc.sync.dma_start(i16b[:], IDX16[:])
        mkb = cpool.tile([128, prep.mtot], BF16, name="mkb")
        nc.sync.dma_start(mkb[:], MASK[:])

        # ---------------- phase 1: layer-1 packed table, replicated
        GRP = 4
        ngrp = npad // (128 * GRP)
        with tc.tile_pool(name="ph1", bufs=3) as ph1, \
             tc.tile_pool(name="ph1p", bufs=8, space="PSUM") as ph1p:
            for g in range(ngrp):
                xsl = ph1.tile([128, 128 * GRP], BF16, tag="xsl")
                nc.sync.dma_start(xsl[:],
                                  XT[:, g * 128 * GRP:(g + 1) * 128 * GRP])
                stage = ph1.tile([128, GRP, FB1], BF16, tag="stage")
                for s in range(GRP):
                    ps = ph1p.tile([128, FB1], F32, tag="ps")
                    nc.tensor.matmul(ps[:], lhsT=xsl[:, s * 128:(s + 1) * 128],
                                     rhs=w1s[:], start=True, stop=True)
                    nc.vector.tensor_copy(stage[:, s, :], ps[:])
                rows = slice(g * 128 * GRP, (g + 1) * 128 * GRP)
                nc.sync.dma_start(
                    t1b[rows, 0:FB1].rearrange("(s p) c -> p s c", p=128),
                    stage[:, :, :])
        # (no pad-row fixups needed: pad slots are killed by the -1e9 masks)

        t1pairs = apd(t1b[:], [[2 * ROW1, cfg.NPAIR], [1, 2 * ROW1]])
        t2pairs = apd(t2b[:], [[2 * ROW2, cfg.NPAIR], [1, 2 * ROW2]])

        # AllGather fires after the group ending at each chunk boundary
        ag_ends = {e: i for i, e in enumerate(cfg.AG_T1)}
        SP = bool(int(os.environ.get("GAT_SP", "0")))
        groups = prep.groups

        # ---------------- phase 2: layer-1 groups -> t2sh shard + chunked AG
        with tc.tile_pool(name="gp", bufs=2) as gp, \
             tc.tile_pool(name="sp", bufs=3) as sp, \
             tc.tile_pool(name="pp", bufs=3, space="PSUM") as pp:
            for gi, (t0, ng, S) in enumerate(groups):
                NS = ng * S                 # slots in group
                K2 = 2 * S
                NK = ng * K2                # slot-members in group
                G = gp.tile([128, NS * 2 * ROW1], BF16, tag="G")
                nc.gpsimd.dma_gather(
                    apv(G[:], [[2 * ROW1, NS], [1, 2 * ROW1]]), t1pairs,
                    i16b[:, GOFS[gi]:GOFS[gi] + NS * 8],
                    NS * 128, NS * 128, 2 * ROW1, single_packet=SP)

                # a_d[dst] per tile from self columns (slot 0), parity-select
                ad0 = sp.tile([128, ng, HEADS], F32, tag="ad0")
                nc.vector.tensor_scalar(
                    ad0[:], apv(G[:], [[S * 2 * ROW1, ng], [1, HEADS]],
                                IN_DIM + HEADS),
                    cb["par0"][:], None, OP.mult)
                ad1 = sp.tile([128, ng, HEADS], F32, tag="ad1")
                nc.vector.tensor_scalar(
                    ad1[:], apv(G[:], [[S * 2 * ROW1, ng], [1, HEADS]],
                                ROW1 + IN_DIM + HEADS),
                    cb["par1"][:], None, OP.mult)
                ad = sp.tile([128, ng, HEADS], F32, tag="ad")
                nc.vector.tensor_tensor(ad[:], ad0[:], ad1[:], OP.add)

                # e = a_s[src] + a_d[dst] + mask ; leaky
                e = sp.tile([128, NK, HEADS], F32, tag="e")
                as_v = apv(G[:], [[ROW1, NK], [1, HEADS]], IN_DIM)
                ad_v = apv(ad[:], [[HEADS, ng], [0, K2], [1, HEADS]])
                nc.vector.tensor_tensor(e[:], as_v, ad_v, OP.add)
                mk_v = apv(mkb[:], [[1, NK], [0, HEADS]], MOFS[gi])
                nc.vector.tensor_tensor(e[:], e[:], mk_v, OP.add)
                e2 = sp.tile([128, NK, HEADS], F32, tag="e2")
                nc.vector.tensor_scalar(e2[:], e[:], NEG, None, OP.mult)
                nc.vector.tensor_tensor(e[:], e[:], e2[:], OP.max)
                # ex = exp(e); den per (tile, head); w = ex/den
                nc.scalar.activation(e[:], e[:], ACT.Exp)
                den = sp.tile([128, ng, HEADS], F32, tag="den")
                nc.vector.reduce_sum(
                    den[:], apv(e[:], [[K2 * HEADS, ng], [1, HEADS],
                                       [HEADS, K2]]), axis=AX.X)
                nc.vector.tensor_scalar(den[:], den[:], 1e-20, None, OP.add)
                inv = sp.tile([128, ng, HEADS], F32, tag="inv")
                nc.vector.reciprocal(inv[:], den[:])
                wb = sp.tile([128, NK, HEADS], BF16, tag="wb")
                inv_v = apv(inv[:], [[HEADS, ng], [0, K2], [1, HEADS]])
                nc.vector.tensor_tensor(wb[:], e[:], inv_v, OP.mult)

                # fold w into gathered h (per member), then aggregate
                for m in range(2):
                    gm = apv(G[:], [[2 * ROW1, NS], [1, IN_DIM]], m * ROW1)
                    wm = apv(wb[:], [[2 * HEADS, NS], [1, HEADS], [0, HID]],
                             m * HEADS)
                    nc.vector.tensor_tensor(gm, gm, wm, OP.mult)
                h1 = sp.tile([128, ng, IN_DIM], F32, tag="h1")
                nc.vector.reduce_sum(
                    h1[:], apv(G[:], [[S * 2 * ROW1, ng], [1, IN_DIM],
                                      [ROW1, K2]]), axis=AX.X)

                # + b1, layernorm, relu (batched over ng tiles)
                b1_v = apv(cb["b1"][:], [[0, ng], [1, IN_DIM]])
                nc.vector.tensor_tensor(h1[:], h1[:], b1_v, OP.add)
                ms = sp.tile([128, ng], F32, tag="ms")
                nc.vector.reduce_sum(ms[:], apv(h1[:], [[IN_DIM, ng],
                                                        [1, IN_DIM]]),
                                     axis=AX.X)
                mu = sp.tile([128, ng], F32, tag="mu")
                nc.vector.tensor_scalar(mu[:], ms[:], 1.0 / IN_DIM, None,
                                        OP.mult)
                mu_v = apv(mu[:], [[1, ng], [0, IN_DIM]])
                nc.vector.tensor_tensor(h1[:], h1[:], mu_v, OP.subtract)
                sq = sp.tile([128, ng, IN_DIM], F32, tag="sq")
                nc.vector.tensor_tensor(sq[:], h1[:], h1[:], OP.mult)
                var = sp.tile([128, ng], F32, tag="var")
                nc.vector.reduce_sum(var[:], apv(sq[:], [[IN_DIM, ng],
                                                         [1, IN_DIM]]),
                                     axis=AX.X)
                std = sp.tile([128, ng], F32, tag="std")
                nc.scalar.activation(std[:], var[:], ACT.Sqrt, bias=epst[:],
                                     scale=1.0 / IN_DIM)
                rstd = sp.tile([128, ng], F32, tag="rstd")
                nc.vector.reciprocal(rstd[:], std[:])
                rstd_v = apv(rstd[:], [[1, ng], [0, IN_DIM]])
                nc.vector.tensor_tensor(h1[:], h1[:], rstd_v, OP.mult)
                g1_v = apv(cb["g1"][:], [[0, ng], [1, IN_DIM]])
                nc.vector.tensor_tensor(h1[:], h1[:], g1_v, OP.mult)
                be1_v = apv(cb["be1"][:], [[0, ng], [1, IN_DIM]])
                nc.vector.tensor_tensor(h1[:], h1[:], be1_v, OP.add)
                nc.vector.tensor_scalar(h1[:], h1[:], 0.0, None, OP.max)

                # layer-2 shard rows: per-tile PE transpose + matmul, batched
                # psum tiles and single copies/DMA per group
                pst = pp.tile([128, ng, 128], F32, tag="pst")
                for i in range(ng):
                    nc.tensor.transpose(pst[:, i, :], h1[:, i, :], ident[:])
                h1t = sp.tile([128, ng, 128], BF16, tag="h1t")
                nc.vector.tensor_copy(h1t[:], pst[:])
                ps2 = pp.tile([128, ng, FB2], F32, tag="ps2")
                for i in range(ng):
                    nc.tensor.matmul(ps2[:, i, :], lhsT=h1t[:, i, :],
                                     rhs=w2s[:], start=True, stop=True)
                t2row = sp.tile([128, ng, FB2], BF16, tag="t2row")
                nc.vector.tensor_copy(t2row[:], ps2[:])
                nc.sync.dma_start(
                    t2sh[t0 * 128:(t0 + ng) * 128, 0:FB2].rearrange(
                        "(s p) c -> p s c", p=128), t2row[:])

                tend = t0 + ng
                if tend in ag_ends:
                    ci = ag_ends[tend]
                    r0 = cfg.AG_T0[ci] * 128
                    r1 = tend * 128
                    ag_out = apd(t2b[:],
                                 [[1, NCORE * (r1 - r0) * ROW2]],
                                 cfg.CH_BASE[ci] * ROW2)
                    nc.gpsimd.collective_compute(
                        "AllGather", OP.bypass,
                        replica_groups=[list(range(NCORE))],
                        ins=[t2sh[r0:r1, :].opt()], outs=[ag_out.opt()])

        # ---------------- phase 3: layer 2
        with tc.tile_pool(name="gp2", bufs=2) as gp2, \
             tc.tile_pool(name="sp2", bufs=3) as sp2:
            for gi, (t0, ng, S) in enumerate(groups):
                NS = ng * S
                K2 = 2 * S
                NK = ng * K2
                G2 = gp2.tile([128, NS * 2 * ROW2], BF16, tag="G2")
                nc.gpsimd.dma_gather(
                    apv(G2[:], [[2 * ROW2, NS], [1, 2 * ROW2]]), t2pairs,
                    i16b[:, GOFS[gi]:GOFS[gi] + NS * 8],
                    NS * 128, NS * 128, 2 * ROW2, single_packet=SP)

                ad0 = sp2.tile([128, ng], F32, tag="ad0_2")
                nc.vector.tensor_scalar(
                    ad0[:], apv(G2[:], [[S * 2 * ROW2, ng]], EMB + 1),
                    cb["par0"][:], None, OP.mult)
                ad1 = sp2.tile([128, ng], F32, tag="ad1_2")
                nc.vector.tensor_scalar(
                    ad1[:], apv(G2[:], [[S * 2 * ROW2, ng]], ROW2 + EMB + 1),
                    cb["par1"][:], None, OP.mult)
                ad = sp2.tile([128, ng], F32, tag="ad_2")
                nc.vector.tensor_tensor(ad[:], ad0[:], ad1[:], OP.add)

                e = sp2.tile([128, NK], F32, tag="e_2")
                as_v = apv(G2[:], [[ROW2, NK]], EMB)
                ad_v = apv(ad[:], [[1, ng], [0, K2]])
                nc.vector.tensor_tensor(e[:], as_v, ad_v, OP.add)
                nc.vector.tensor_tensor(e[:], e[:],
                                        apv(mkb[:], [[1, NK]], MOFS[gi]),
                                        OP.add)
                e2 = sp2.tile([128, NK], F32, tag="e2_2")
                nc.vector.tensor_scalar(e2[:], e[:], NEG, None, OP.mult)
                nc.vector.tensor_tensor(e[:], e[:], e2[:], OP.max)
                nc.scalar.activation(e[:], e[:], ACT.Exp)
                den = sp2.tile([128, ng], F32, tag="den2")
                nc.vector.reduce_sum(den[:], apv(e[:], [[K2, ng], [1, K2]]),
                                     axis=AX.X)
                nc.vector.tensor_scalar(den[:], den[:], 1e-20, None, OP.add)
                inv = sp2.tile([128, ng], F32, tag="inv2")
                nc.vector.reciprocal(inv[:], den[:])
                wb = sp2.tile([128, NK], BF16, tag="wb2")
                inv_v = apv(inv[:], [[1, ng], [0, K2]])
                nc.vector.tensor_tensor(wb[:], e[:], inv_v, OP.mult)

                for m in range(2):
                    gm = apv(G2[:], [[2 * ROW2, NS], [1, EMB]], m * ROW2)
                    wm = apv(wb[:], [[2, NS], [0, EMB]], m)
                    nc.vector.tensor_tensor(gm, gm, wm, OP.mult)
                h2 = sp2.tile([128, ng, EMB], F32, tag="h2")
                nc.vector.reduce_sum(
                    h2[:], apv(G2[:], [[S * 2 * ROW2, ng], [1, EMB],
                                       [ROW2, K2]]), axis=AX.X)

                b2_v = apv(cb["b2"][:], [[0, ng], [1, EMB]])
                nc.vector.tensor_tensor(h2[:], h2[:], b2_v, OP.add)
                ms = sp2.tile([128, ng], F32, tag="ms2")
                nc.vector.reduce_sum(ms[:], apv(h2[:], [[EMB, ng], [1, EMB]]),
                                     axis=AX.X)
                mu = sp2.tile([128, ng], F32, tag="mu2")
                nc.vector.tensor_scalar(mu[:], ms[:], 1.0 / EMB, None, OP.mult)
                mu_v = apv(mu[:], [[1, ng], [0, EMB]])
                nc.vector.tensor_tensor(h2[:], h2[:], mu_v, OP.subtract)
                sq = sp2.tile([128, ng, EMB], F32, tag="sq2")
                nc.vector.tensor_tensor(sq[:], h2[:], h2[:], OP.mult)
                var = sp2.tile([128, ng], F32, tag="var2")
                nc.vector.reduce_sum(var[:], apv(sq[:], [[EMB, ng], [1, EMB]]),
                                     axis=AX.X)
                std = sp2.tile([128, ng], F32, tag="std2")
                nc.scalar.activation(std[:], var[:], ACT.Sqrt, bias=epst[:],
                                     scale=1.0 / EMB)
                rstd = sp2.tile([128, ng], F32, tag="rstd2")
                nc.vector.reciprocal(rstd[:], std[:])
                rstd_v = apv(rstd[:], [[1, ng], [0, EMB]])
                nc.vector.tensor_tensor(h2[:], h2[:], rstd_v, OP.mult)
                g2_v = apv(cb["g2"][:], [[0, ng], [1, EMB]])
                nc.vector.tensor_tensor(h2[:], h2[:], g2_v, OP.mult)
                be2_v = apv(cb["be2"][:], [[0, ng], [1, EMB]])
                nc.vector.tensor_tensor(h2[:], h2[:], be2_v, OP.add)
                nc.sync.dma_start(
                    OUT[t0 * 128:(t0 + ng) * 128, :].rearrange(
                        "(s p) c -> p s c", p=128), h2[:])

    nc.compile()
    return nc


# ------------------------------------------------------------------ execution
def make_in_maps(cfg, prep, inputs):
    wts = host_weights(cfg, inputs)
    xt = host_xt(cfg, prep, inputs["x"])
    in_maps = []
    for c in range(NCORE):
        m = {"xt": xt,
             "idx16": prep.idx16[c],
             "mask": prep.mask[c]}
        m.update(wts)
        in_maps.append(m)
    return in_maps


def assemble(cfg, prep, outs):
    full = np.zeros((cfg.N, EMB), np.float32)
    for c in range(NCORE):
        o = outs[c]["out"]
        olds = prep.new2old[c * cfg.NPC:(c + 1) * cfg.NPC]
        valid = olds >= 0
        full[olds[valid]] = o[valid]
    return full


_CACHE = {}


def kernel(**inputs):
    from concourse.bass_utils import run_bass_kernel_spmd
    cfg = CFG
    edge_index = np.asarray(inputs["edge_index"])
    if "prog" not in _CACHE:
        prep = host_prep(cfg, edge_index)
        nc = build_program(cfg, prep)
        _CACHE["prog"] = (prep, nc)
    prep, nc = _CACHE["prog"]
    in_maps = make_in_maps(cfg, prep, inputs)
    res = run_bass_kernel_spmd(
        nc, in_maps, core_ids=list(range(NCORE)),
        trace=bool(int(os.environ.get("GAT_TRACE", "0"))))
    out = assemble(cfg, prep, res.results)
    if res.exec_time_ns is not None:
        kernel.last_exec_time_ns = res.exec_time_ns
    return out


kernel.last_exec_time_ns = None


# revision 26
# speedup vs baseline: 1.1643x; 1.1643x over previous
"""2-layer GAT (GATConv x2 + LayerNorm + ReLU) on Trainium2, 8-core SPMD.

v2 design (vs baseline): pair-gather edge streams.
  - Nodes degree-sorted (incl self loop), dealt round-robin across 8 cores;
    each core owns NPC=6272 dst slots (49 tiles x 128).
  - Per layer a packed bf16 node table in DRAM:
      layer 1: row = [h(128) | a_s(4) | a_d(4) | pad] -> 192 cols (384B)
      layer 2: row = [h2(64) | a_s2(1) | a_d2(1) | pad] -> 128 cols (256B)
    Rows are gathered in PAIRS (idx = row>>1, elem = 2 rows) so a single
    int16 gather covers all 50176 rows (25088 pair indices < 32768): ONE
    dma_gather per dst tile per layer, real edges only (~1.06x padding
    vs 1.86x for the per-half slot-padded baseline).
  - Per dst tile [128 dst x K slots x 2 pair-members]: slot 0 is the self
    loop; host-built bf16 masks (-1e9 on the wrong pair member / pad slots)
    zero out garbage in the softmax.  e-scores are bounded (|e| <= ~6) so
    the segment max is skipped; weights w = exp(leaky(e))/den are folded
    into the gathered h before a single strided DVE reduction.
  - Layer-2 tables AllGathered in 4 chunks (Shared scratchpad output)
    overlapped with the layer-1 tail.
"""

import os
import types
from contextlib import ExitStack

import numpy as np

import concourse.bass as bass
import concourse.mybir as mybir
import concourse.tile as tile
from concourse import bacc
from concourse.bass import AP
from concourse.masks import make_identity

F32 = mybir.dt.float32
BF16 = mybir.dt.bfloat16
I16 = mybir.dt.int16
AX = mybir.AxisListType
OP = mybir.AluOpType
ACT = mybir.ActivationFunctionType

# ---------------------------------------------------------------- problem cfg
N = 50000
E = 800000
IN_DIM = 128
HID = 32
HEADS = 4
EMB = 64
NEG = 0.2
EPS = 1e-5
NCORE = 8
ROW1 = 192        # layer-1 packed table row (bf16 cols): h(128) as(4) ad(4) pad
ROW2 = 128        # layer-2 packed table row: h2(64) as(1) ad(1) pad
NEGBIG = -1e9
AG_CHUNKS = 4


def make_cfg(n_nodes, tiles_per_core):
    c = types.SimpleNamespace()
    c.N = n_nodes
    c.TILES = tiles_per_core
    c.NPC = tiles_per_core * 128
    c.NPAD = NCORE * c.NPC
    c.NPAIR = c.NPAD // 2
    c.REAL_PC = n_nodes // NCORE
    # AllGather chunk geometry (tile index ranges per chunk)
    ends = [((i + 1) * tiles_per_core) // AG_CHUNKS for i in range(AG_CHUNKS)]
    starts = [0] + ends[:-1]
    c.AG_T0 = starts
    c.AG_T1 = ends
    # table rows are stored [chunk, core, rows] so each chunked AllGather
    # output region is contiguous; srow maps (core, pos) -> storage row
    rs = [s * 128 for s in starts]
    re = [e * 128 for e in ends]
    c.CH_ROWS = [b - a for a, b in zip(rs, re)]
    c.CH_BASE = [NCORE * a for a in rs]
    pos = np.arange(c.NPC)
    k = np.searchsorted(np.asarray(re), pos, side="right")
    c.SROW = np.stack([
        np.asarray(c.CH_BASE)[k] + cc * np.asarray(c.CH_ROWS)[k]
        + (pos - np.asarray(rs)[k]) for cc in range(NCORE)])  # [core, pos]
    assert c.REAL_PC % 2 == 0 and c.REAL_PC < c.NPC and c.NPAIR <= 32768
    return c


CFG = make_cfg(N, 49)


# ------------------------------------------------------------------ host prep
def host_prep(cfg, edge_index):
    """Node permutation, per-tile slot counts, idx16 pair stream and masks.

    Per (core, tile t) the gather stream is K[t] columns of 128 slots:
    column 0 = self loops, columns 1.. = neighbors (dst-grouped), pads point
    at the PADPAIR rows.  idx values are PAIR indices (row>>1); the bf16
    mask [128, 2*K[t]] holds 0 on the real pair member and -1e9 elsewhere.
    """
    n, npc, tiles = cfg.N, cfg.NPC, cfg.TILES
    src = np.asarray(edge_index[0], np.int64)
    dst = np.asarray(edge_index[1], np.int64)

    deg = np.bincount(dst, minlength=n) + 1           # incl self loop
    order = np.argsort(-deg, kind="stable")
    newid = np.empty(n, np.int64)
    r = np.arange(n)
    newid[order] = (r % NCORE) * npc + (r // NCORE)
    new2old = np.full(NCORE * npc, -1, np.int64)
    new2old[newid] = np.arange(n)

    degs_sorted = np.zeros(tiles * 1024, np.int64)
    degs_sorted[:n] = deg[order]
    K = degs_sorted.reshape(tiles, 1024).max(axis=1)
    K = np.maximum(K, 1).astype(np.int64)             # slots per tile

    # group consecutive tiles (equal slot count S per group) for batched
    # gathers + DVE ops; n*S <= GCAP slots, n <= 4 (psum bank limit)
    GCAP = 48
    ag_ends = set(cfg.AG_T1)
    groups = []
    t = 0
    while t < tiles:
        S = int(K[t])
        ng = 1
        while (t + ng < tiles and ng < 4
               and (t + ng) not in ag_ends
               and (ng + 1) * max(S, int(K[t + ng])) <= GCAP):
            S = max(S, int(K[t + ng]))
            ng += 1
        groups.append((t, ng, S))
        t += ng
    gslots = [ng * S for (_, ng, S) in groups]
    GOFS = [0]
    for gs in gslots:
        GOFS.append(GOFS[-1] + gs * 128)              # idx stream offsets
    MOFS_G = [0]
    for gs in gslots:
        MOFS_G.append(MOFS_G[-1] + gs * 2)            # mask col offsets
    total_idx = GOFS[-1]
    mtot = MOFS_G[-1]
    # per-tile lookup: group id, index within group, group S
    g_of_t = np.zeros(tiles, np.int64)
    i_of_t = np.zeros(tiles, np.int64)
    S_of_t = np.zeros(tiles, np.int64)
    for gi, (t0, ng, S) in enumerate(groups):
        g_of_t[t0:t0 + ng] = gi
        i_of_t[t0:t0 + ng] = np.arange(ng)
        S_of_t[t0:t0 + ng] = S
    GOFS_np = np.asarray(GOFS[:-1])
    MOFS_np = np.asarray(MOFS_G[:-1])

    tl = cfg.SROW.reshape(-1)              # logical row -> storage row
    padpair = int(tl[cfg.REAL_PC]) >> 1    # core-0 pad rows (zero features)
    assert int(tl[cfg.REAL_PC]) % 2 == 0

    ns, nd = newid[src], newid[dst]
    eo = np.argsort(nd, kind="stable")
    snd, sns = nd[eo], ns[eo]
    starts = np.r_[0, np.flatnonzero(np.diff(snd)) + 1]
    runlen = np.diff(np.r_[starts, len(snd)])
    runpos = np.arange(len(snd)) - np.repeat(starts, runlen)
    slot = runpos + 1                                  # col 0 = self
    cs = snd // npc
    pos = snd % npc
    ts_ = pos // 128
    ps_ = pos % 128
    assert (slot < K[ts_]).all()

    srows = tl[sns]
    idx16 = np.full((NCORE, total_idx), padpair, np.int16)
    mask = np.full((NCORE, 128, mtot), NEGBIG, np.float32)
    scol = i_of_t[ts_] * S_of_t[ts_] + slot           # slot col within group
    posi = GOFS_np[g_of_t[ts_]] + scol * 128 + ps_
    idx16[cs, posi] = (srows >> 1).astype(np.int16)
    mask[cs, ps_, MOFS_np[g_of_t[ts_]] + scol * 2 + (srows & 1)] = 0.0

    # self column (slot 0 of each tile) per core
    for c in range(NCORE):
        own = c * npc + np.arange(npc)
        valid = new2old[own] >= 0
        t_all = np.arange(npc) // 128
        p_all = np.arange(npc) % 128
        sc0 = i_of_t[t_all] * S_of_t[t_all]
        sown = tl[own]
        idx16[c, GOFS_np[g_of_t[t_all[valid]]] + sc0[valid] * 128
              + p_all[valid]] = (sown[valid] >> 1).astype(np.int16)
        mask[c, p_all[valid], MOFS_np[g_of_t[t_all[valid]]] + sc0[valid] * 2
             + (sown[valid] & 1)] = 0.0

    w = idx16.reshape(NCORE, total_idx // 16, 16).transpose(0, 2, 1)
    idx16_w = np.ascontiguousarray(np.tile(w, (1, 8, 1)))
    return types.SimpleNamespace(
        new2old=new2old, newid=newid,
        K=[int(v) for v in K], groups=groups,
        GOFS=[v // 16 for v in GOFS], MOFS=MOFS_G,
        c16=total_idx // 16, mtot=mtot,
        idx16=idx16_w, mask=_bf16(mask),
    )


def _bf16(x):
    import ml_dtypes
    x = np.ascontiguousarray(np.asarray(x, np.float32))
    u = x.view(np.uint32)
    r = ((u + 0x7FFF + ((u >> 16) & 1)) >> 16).astype(np.uint16)
    return r.view(ml_dtypes.bfloat16)


def host_weights(cfg, inputs):
    W1 = np.asarray(inputs["W1"], np.float32)
    W2 = np.asarray(inputs["W2"], np.float32)
    as1 = np.asarray(inputs["att_src1"], np.float32)
    ad1 = np.asarray(inputs["att_dst1"], np.float32)
    as2 = np.asarray(inputs["att_src2"], np.float32)
    ad2 = np.asarray(inputs["att_dst2"], np.float32)
    W1r = W1.reshape(IN_DIM, HEADS, HID)
    w_as1 = np.einsum("fhc,hc->fh", W1r, as1)
    w_ad1 = np.einsum("fhc,hc->fh", W1r, ad1)
    W1ext = np.concatenate([W1, w_as1, w_ad1], axis=1)            # [128,136]
    W2ext = np.concatenate([W2, W2 @ as2[0][:, None], W2 @ ad2[0][:, None]],
                           axis=1)                                # [128,66]
    par0 = ((np.arange(128) + 1) % 2).astype(np.float32)[:, None]
    par1 = (np.arange(128) % 2).astype(np.float32)[:, None]
    return {
        "w1ext": _bf16(W1ext), "w2ext": _bf16(W2ext),
        "b1": np.tile(np.asarray(inputs["b1"], np.float32), (128, 1)),
        "g1": np.tile(np.asarray(inputs["gamma1"], np.float32), (128, 1)),
        "be1": np.tile(np.asarray(inputs["beta1"], np.float32), (128, 1)),
        "b2": np.tile(np.asarray(inputs["b2"], np.float32), (128, 1)),
        "g2": np.tile(np.asarray(inputs["gamma2"], np.float32), (128, 1)),
        "be2": np.tile(np.asarray(inputs["beta2"], np.float32), (128, 1)),
        "par0": par0, "par1": par1,
    }


def host_xt(cfg, prep, x):
    tl = cfg.SROW.reshape(-1)
    xt = np.zeros((IN_DIM, cfg.NPAD), np.float32)
    xt[:, tl[prep.newid]] = np.asarray(x, np.float32).T
    return _bf16(xt)


# ----------------------------------------------------------------- AP helpers
def apv(ap: AP, dims, extra_offset=0):
    """Replace the free dims of `ap` with explicit [step, count] pairs."""
    return AP(ap.tensor, int(ap.offset + extra_offset),
              [list(ap.ap[0])] + [[int(s), int(n)] for s, n in dims])


def apd(ap: AP, dims, extra_offset=0):
    """DRAM AP with explicit dims (no partition dim)."""
    return AP(ap.tensor, int(ap.offset + extra_offset),
              [[int(s), int(n)] for s, n in dims])


# ------------------------------------------------------------- device program
def build_program(cfg, prep):
    nc = bacc.Bacc("TRN2", target_bir_lowering=False, debug=False,
                   num_devices=NCORE)
    tiles, npc, npad = cfg.TILES, cfg.NPC, cfg.NPAD
    GOFS, MOFS = prep.GOFS, prep.MOFS
    FB1 = IN_DIM + 2 * HEADS            # 136
    FB2 = EMB + 2                       # 66

    XT = nc.dram_tensor("xt", [IN_DIM, npad], BF16, kind="ExternalInput")
    W1e = nc.dram_tensor("w1ext", [IN_DIM, FB1], BF16, kind="ExternalInput")
    W2e = nc.dram_tensor("w2ext", [IN_DIM, FB2], BF16, kind="ExternalInput")
    IDX16 = nc.dram_tensor("idx16", [128, prep.c16], I16, kind="ExternalInput")
    MASK = nc.dram_tensor("mask", [128, prep.mtot], BF16, kind="ExternalInput")
    CB = {}
    for nm, cols in [("b1", IN_DIM), ("g1", IN_DIM), ("be1", IN_DIM),
                     ("b2", EMB), ("g2", EMB), ("be2", EMB),
                     ("par0", 1), ("par1", 1)]:
        CB[nm] = nc.dram_tensor(nm, [128, cols], F32, kind="ExternalInput")
    OUT = nc.dram_tensor("out", [npc, EMB], F32, kind="ExternalOutput")

    with tile.TileContext(nc, num_cores=NCORE) as tc, ExitStack() as ctx:
        dram = ctx.enter_context(tc.tile_pool(name="dram", bufs=1,
                                              space="DRAM"))
        t1b = dram.tile([npad, ROW1], BF16, name="t1b")
        t2sh = dram.tile([npc, ROW2], BF16, name="t2sh")
        t2b = dram.tile([npad, ROW2], BF16, name="t2b")

        cpool = ctx.enter_context(tc.tile_pool(name="const", bufs=1))
        w1s = cpool.tile([IN_DIM, FB1], BF16, name="w1s")
        w2s = cpool.tile([IN_DIM, FB2], BF16, name="w2s")
        nc.sync.dma_start(w1s[:], W1e[:])
        nc.sync.dma_start(w2s[:], W2e[:])
        cb = {}
        for nm in CB:
            cb[nm] = cpool.tile(list(CB[nm].shape), F32, name=f"sb_{nm}")
            nc.sync.dma_start(cb[nm][:], CB[nm][:])
        ident = cpool.tile([128, 128], F32, name="ident")
        make_identity(nc, ident[:])
        epst = cpool.tile([128, 1], F32, name="epst")
        nc.vector.memset(epst[:], EPS)
        i16b = cpool.tile([128, prep.c16], I16, name="i16b")
        nc.sync.dma_start(i16b[:], IDX16[:])
        mkb = cpool.tile([128, prep.mtot], BF16, name="mkb")
        nc.sync.dma_start(mkb[:], MASK[:])

        # ---------------- phase 1: layer-1 packed table, replicated
        GRP = 4
        ngrp = npad // (128 * GRP)
        with tc.tile_pool(name="ph1", bufs=3) as ph1, \
             tc.tile_pool(name="ph1p", bufs=8, space="PSUM") as ph1p:
            for g in range(ngrp):
                xsl = ph1.tile([128, 128 * GRP], BF16, tag="xsl")
                nc.sync.dma_start(xsl[:],
                                  XT[:, g * 128 * GRP:(g + 1) * 128 * GRP])
                # full-ROW1 rows so the t1b write is one contiguous region
                stage = ph1.tile([128, GRP, ROW1], BF16, tag="stage")
                for s in range(GRP):
                    ps = ph1p.tile([128, FB1], F32, tag="ps")
                    nc.tensor.matmul(ps[:], lhsT=xsl[:, s * 128:(s + 1) * 128],
                                     rhs=w1s[:], start=True, stop=True)
                    nc.vector.tensor_copy(stage[:, s, 0:FB1], ps[:])
                rows = slice(g * 128 * GRP, (g + 1) * 128 * GRP)
                nc.sync.dma_start(
                    t1b[rows, :].rearrange("(s p) c -> p s c", p=128),
                    stage[:, :, :])
        # (no pad-row fixups needed: pad slots are killed by the -1e9 masks)

        t1pairs = apd(t1b[:], [[2 * ROW1, cfg.NPAIR], [1, 2 * ROW1]])
        t2pairs = apd(t2b[:], [[2 * ROW2, cfg.NPAIR], [1, 2 * ROW2]])

        # AllGather fires after the group ending at each chunk boundary
        ag_ends = {e: i for i, e in enumerate(cfg.AG_T1)}
        SP = bool(int(os.environ.get("GAT_SP", "0")))
        groups = prep.groups

        # ---------------- phase 2: layer-1 groups -> t2sh shard + chunked AG
        with tc.tile_pool(name="gp", bufs=3) as gp, \
             tc.tile_pool(name="sp", bufs=3) as sp, \
             tc.tile_pool(name="pp", bufs=3, space="PSUM") as pp:
            for gi, (t0, ng, S) in enumerate(groups):
                NS = ng * S                 # slots in group
                K2 = 2 * S
                NK = ng * K2                # slot-members in group
                G = gp.tile([128, NS * 2 * ROW1], BF16, tag="G")
                nc.gpsimd.dma_gather(
                    apv(G[:], [[2 * ROW1, NS], [1, 2 * ROW1]]), t1pairs,
                    i16b[:, GOFS[gi]:GOFS[gi] + NS * 8],
                    NS * 128, NS * 128, 2 * ROW1, single_packet=SP)

                # a_d[dst] per tile from self columns (slot 0), parity-select
                ad0 = sp.tile([128, ng, HEADS], F32, tag="ad0")
                nc.vector.tensor_scalar(
                    ad0[:], apv(G[:], [[S * 2 * ROW1, ng], [1, HEADS]],
                                IN_DIM + HEADS),
                    cb["par0"][:], None, OP.mult)
                ad1 = sp.tile([128, ng, HEADS], F32, tag="ad1")
                nc.vector.tensor_scalar(
                    ad1[:], apv(G[:], [[S * 2 * ROW1, ng], [1, HEADS]],
                                ROW1 + IN_DIM + HEADS),
                    cb["par1"][:], None, OP.mult)
                ad = sp.tile([128, ng, HEADS], F32, tag="ad")
                nc.vector.tensor_tensor(ad[:], ad0[:], ad1[:], OP.add)

                # e = a_s[src] + a_d[dst] + mask ; leaky
                e = sp.tile([128, NK, HEADS], F32, tag="e")
                as_v = apv(G[:], [[ROW1, NK], [1, HEADS]], IN_DIM)
                ad_v = apv(ad[:], [[HEADS, ng], [0, K2], [1, HEADS]])
                nc.vector.tensor_tensor(e[:], as_v, ad_v, OP.add)
                mk_v = apv(mkb[:], [[1, NK], [0, HEADS]], MOFS[gi])
                nc.vector.tensor_tensor(e[:], e[:], mk_v, OP.add)
                e2 = sp.tile([128, NK, HEADS], F32, tag="e2")
                nc.vector.tensor_scalar(e2[:], e[:], NEG, None, OP.mult)
                nc.vector.tensor_tensor(e[:], e[:], e2[:], OP.max)
                # ex = exp(e) (bf16, unnormalized); den per (tile, head)
                exb = sp.tile([128, NK, HEADS], BF16, tag="exb")
                nc.scalar.activation(exb[:], e[:], ACT.Exp)
                den = sp.tile([128, ng, HEADS], F32, tag="den")
                nc.vector.reduce_sum(
                    den[:], apv(exb[:], [[K2 * HEADS, ng], [1, HEADS],
                                         [HEADS, K2]]), axis=AX.X)
                nc.vector.tensor_scalar(den[:], den[:], 1e-20, None, OP.add)
                inv = sp.tile([128, ng, HEADS], F32, tag="inv")
                nc.vector.reciprocal(inv[:], den[:])

                # fold ex into gathered h (per member); tree-halve the slot
                # members (bf16) then a short f32 strided reduce; divide by
                # den after aggregation
                for m in range(2):
                    gm = apv(G[:], [[2 * ROW1, NS], [1, IN_DIM]], m * ROW1)
                    wm = apv(exb[:], [[2 * HEADS, NS], [1, HEADS], [0, HID]],
                             m * HEADS)
                    nc.vector.tensor_tensor(gm, gm, wm, OP.mult)
                n = K2
                for _ in range(2):
                    hh = (n + 1) // 2
                    dst = apv(G[:], [[S * 2 * ROW1, ng], [ROW1, n - hh],
                                     [1, IN_DIM]])
                    src = apv(G[:], [[S * 2 * ROW1, ng], [ROW1, n - hh],
                                     [1, IN_DIM]], hh * ROW1)
                    nc.vector.tensor_tensor(dst, dst, src, OP.add)
                    n = hh
                h1 = sp.tile([128, ng, IN_DIM], F32, tag="h1")
                nc.vector.reduce_sum(
                    h1[:], apv(G[:], [[S * 2 * ROW1, ng], [1, IN_DIM],
                                      [ROW1, n]]), axis=AX.X)
                inv_v = apv(inv[:], [[HEADS, ng], [1, HEADS], [0, HID]])
                nc.vector.tensor_tensor(h1[:], h1[:], inv_v, OP.mult)

                # + b1, layernorm, relu (batched over ng tiles)
                b1_v = apv(cb["b1"][:], [[0, ng], [1, IN_DIM]])
                nc.vector.tensor_tensor(h1[:], h1[:], b1_v, OP.add)
                ms = sp.tile([128, ng], F32, tag="ms")
                nc.vector.reduce_sum(ms[:], apv(h1[:], [[IN_DIM, ng],
                                                        [1, IN_DIM]]),
                                     axis=AX.X)
                mu = sp.tile([128, ng], F32, tag="mu")
                nc.vector.tensor_scalar(mu[:], ms[:], 1.0 / IN_DIM, None,
                                        OP.mult)
                mu_v = apv(mu[:], [[1, ng], [0, IN_DIM]])
                nc.vector.tensor_tensor(h1[:], h1[:], mu_v, OP.subtract)
                sq = sp.tile([128, ng, IN_DIM], F32, tag="sq")
                nc.vector.tensor_tensor(sq[:], h1[:], h1[:], OP.mult)
                var = sp.tile([128, ng], F32, tag="var")
                nc.vector.reduce_sum(var[:], apv(sq[:], [[IN_DIM, ng],
                                                         [1, IN_DIM]]),
                                     axis=AX.X)
                std = sp.tile([128, ng], F32, tag="std")
                nc.scalar.activation(std[:], var[:], ACT.Sqrt, bias=epst[:],
                                     scale=1.0 / IN_DIM)
                rstd = sp.tile([128, ng], F32, tag="rstd")
                nc.vector.reciprocal(rstd[:], std[:])
                rstd_v = apv(rstd[:], [[1, ng], [0, IN_DIM]])
                nc.vector.tensor_tensor(h1[:], h1[:], rstd_v, OP.mult)
                g1_v = apv(cb["g1"][:], [[0, ng], [1, IN_DIM]])
                nc.vector.tensor_tensor(h1[:], h1[:], g1_v, OP.mult)
                be1_v = apv(cb["be1"][:], [[0, ng], [1, IN_DIM]])
                nc.vector.tensor_tensor(h1[:], h1[:], be1_v, OP.add)
                nc.vector.tensor_scalar(h1[:], h1[:], 0.0, None, OP.max)

                # layer-2 shard rows: per-tile PE transpose + matmul, batched
                # psum tiles and single copies/DMA per group
                pst = pp.tile([128, ng, 128], F32, tag="pst")
                for i in range(ng):
                    nc.tensor.transpose(pst[:, i, :], h1[:, i, :], ident[:])
                h1t = sp.tile([128, ng, 128], BF16, tag="h1t")
                nc.vector.tensor_copy(h1t[:], pst[:])
                ps2 = pp.tile([128, ng, FB2], F32, tag="ps2")
                for i in range(ng):
                    nc.tensor.matmul(ps2[:, i, :], lhsT=h1t[:, i, :],
                                     rhs=w2s[:], start=True, stop=True)
                t2row = sp.tile([128, ng, FB2], BF16, tag="t2row")
                nc.vector.tensor_copy(t2row[:], ps2[:])
                nc.sync.dma_start(
                    t2sh[t0 * 128:(t0 + ng) * 128, 0:FB2].rearrange(
                        "(s p) c -> p s c", p=128), t2row[:])

                tend = t0 + ng
                if tend in ag_ends:
                    ci = ag_ends[tend]
                    r0 = cfg.AG_T0[ci] * 128
                    r1 = tend * 128
                    ag_out = apd(t2b[:],
                                 [[1, NCORE * (r1 - r0) * ROW2]],
                                 cfg.CH_BASE[ci] * ROW2)
                    nc.gpsimd.collective_compute(
                        "AllGather", OP.bypass,
                        replica_groups=[list(range(NCORE))],
                        ins=[t2sh[r0:r1, :].opt()], outs=[ag_out.opt()])

        # ---------------- phase 3: layer 2
        with tc.tile_pool(name="gp2", bufs=3) as gp2, \
             tc.tile_pool(name="sp2", bufs=3) as sp2:
            for gi, (t0, ng, S) in enumerate(groups):
                NS = ng * S
                K2 = 2 * S
                NK = ng * K2
                G2 = gp2.tile([128, NS * 2 * ROW2], BF16, tag="G2")
                nc.gpsimd.dma_gather(
                    apv(G2[:], [[2 * ROW2, NS], [1, 2 * ROW2]]), t2pairs,
                    i16b[:, GOFS[gi]:GOFS[gi] + NS * 8],
                    NS * 128, NS * 128, 2 * ROW2, single_packet=SP)

                ad0 = sp2.tile([128, ng], F32, tag="ad0_2")
                nc.vector.tensor_scalar(
                    ad0[:], apv(G2[:], [[S * 2 * ROW2, ng]], EMB + 1),
                    cb["par0"][:], None, OP.mult)
                ad1 = sp2.tile([128, ng], F32, tag="ad1_2")
                nc.vector.tensor_scalar(
                    ad1[:], apv(G2[:], [[S * 2 * ROW2, ng]], ROW2 + EMB + 1),
                    cb["par1"][:], None, OP.mult)
                ad = sp2.tile([128, ng], F32, tag="ad_2")
                nc.vector.tensor_tensor(ad[:], ad0[:], ad1[:], OP.add)

                e = sp2.tile([128, NK], F32, tag="e_2")
                as_v = apv(G2[:], [[ROW2, NK]], EMB)
                ad_v = apv(ad[:], [[1, ng], [0, K2]])
                nc.vector.tensor_tensor(e[:], as_v, ad_v, OP.add)
                nc.vector.tensor_tensor(e[:], e[:],
                                        apv(mkb[:], [[1, NK]], MOFS[gi]),
                                        OP.add)
                e2 = sp2.tile([128, NK], F32, tag="e2_2")
                nc.vector.tensor_scalar(e2[:], e[:], NEG, None, OP.mult)
                nc.vector.tensor_tensor(e[:], e[:], e2[:], OP.max)
                exb = sp2.tile([128, NK], BF16, tag="exb2")
                nc.scalar.activation(exb[:], e[:], ACT.Exp)
                den = sp2.tile([128, ng], F32, tag="den2")
                nc.vector.reduce_sum(den[:], apv(exb[:], [[K2, ng], [1, K2]]),
                                     axis=AX.X)
                nc.vector.tensor_scalar(den[:], den[:], 1e-20, None, OP.add)
                inv = sp2.tile([128, ng], F32, tag="inv2")
                nc.vector.reciprocal(inv[:], den[:])

                for m in range(2):
                    gm = apv(G2[:], [[2 * ROW2, NS], [1, EMB]], m * ROW2)
                    wm = apv(exb[:], [[2, NS], [0, EMB]], m)
                    nc.vector.tensor_tensor(gm, gm, wm, OP.mult)
                n = K2
                for _ in range(2):
                    hh = (n + 1) // 2
                    dst = apv(G2[:], [[S * 2 * ROW2, ng], [ROW2, n - hh],
                                      [1, EMB]])
                    src = apv(G2[:], [[S * 2 * ROW2, ng], [ROW2, n - hh],
                                      [1, EMB]], hh * ROW2)
                    nc.vector.tensor_tensor(dst, dst, src, OP.add)
                    n = hh
                h2 = sp2.tile([128, ng, EMB], F32, tag="h2")
                nc.vector.reduce_sum(
                    h2[:], apv(G2[:], [[S * 2 * ROW2, ng], [1, EMB],
                                       [ROW2, n]]), axis=AX.X)
                inv_v = apv(inv[:], [[1, ng], [0, EMB]])
                nc.vector.tensor_tensor(h2[:], h2[:], inv_v, OP.mult)

                b2_v = apv(cb["b2"][:], [[0, ng], [1, EMB]])
                nc.vector.tensor_tensor(h2[:], h2[:], b2_v, OP.add)
                ms = sp2.tile([128, ng], F32, tag="ms2")
                nc.vector.reduce_sum(ms[:], apv(h2[:], [[EMB, ng], [1, EMB]]),
                                     axis=AX.X)
                mu = sp2.tile([128, ng], F32, tag="mu2")
                nc.vector.tensor_scalar(mu[:], ms[:], 1.0 / EMB, None, OP.mult)
                mu_v = apv(mu[:], [[1, ng], [0, EMB]])
                nc.vector.tensor_tensor(h2[:], h2[:], mu_v, OP.subtract)
                sq = sp2.tile([128, ng, EMB], F32, tag="sq2")
                nc.vector.tensor_tensor(sq[:], h2[:], h2[:], OP.mult)
                var = sp2.tile([128, ng], F32, tag="var2")
                nc.vector.reduce_sum(var[:], apv(sq[:], [[EMB, ng], [1, EMB]]),
                                     axis=AX.X)
                std = sp2.tile([128, ng], F32, tag="std2")
                nc.scalar.activation(std[:], var[:], ACT.Sqrt, bias=epst[:],
                                     scale=1.0 / EMB)
                rstd = sp2.tile([128, ng], F32, tag="rstd2")
                nc.vector.reciprocal(rstd[:], std[:])
                rstd_v = apv(rstd[:], [[1, ng], [0, EMB]])
                nc.vector.tensor_tensor(h2[:], h2[:], rstd_v, OP.mult)
                g2_v = apv(cb["g2"][:], [[0, ng], [1, EMB]])
                nc.vector.tensor_tensor(h2[:], h2[:], g2_v, OP.mult)
                be2_v = apv(cb["be2"][:], [[0, ng], [1, EMB]])
                nc.vector.tensor_tensor(h2[:], h2[:], be2_v, OP.add)
                nc.sync.dma_start(
                    OUT[t0 * 128:(t0 + ng) * 128, :].rearrange(
                        "(s p) c -> p s c", p=128), h2[:])

    nc.compile()
    return nc


# ------------------------------------------------------------------ execution
def make_in_maps(cfg, prep, inputs):
    wts = host_weights(cfg, inputs)
    xt = host_xt(cfg, prep, inputs["x"])
    in_maps = []
    for c in range(NCORE):
        m = {"xt": xt,
             "idx16": prep.idx16[c],
             "mask": prep.mask[c]}
        m.update(wts)
        in_maps.append(m)
    return in_maps


def assemble(cfg, prep, outs):
    full = np.zeros((cfg.N, EMB), np.float32)
    for c in range(NCORE):
        o = outs[c]["out"]
        olds = prep.new2old[c * cfg.NPC:(c + 1) * cfg.NPC]
        valid = olds >= 0
        full[olds[valid]] = o[valid]
    return full


_CACHE = {}


def kernel(**inputs):
    from concourse.bass_utils import run_bass_kernel_spmd
    cfg = CFG
    edge_index = np.asarray(inputs["edge_index"])
    if "prog" not in _CACHE:
        prep = host_prep(cfg, edge_index)
        nc = build_program(cfg, prep)
        _CACHE["prog"] = (prep, nc)
    prep, nc = _CACHE["prog"]
    in_maps = make_in_maps(cfg, prep, inputs)
    res = run_bass_kernel_spmd(
        nc, in_maps, core_ids=list(range(NCORE)),
        trace=bool(int(os.environ.get("GAT_TRACE", "0"))))
    out = assemble(cfg, prep, res.results)
    if res.exec_time_ns is not None:
        kernel.last_exec_time_ns = res.exec_time_ns
    return out


kernel.last_exec_time_ns = None
